# revision 37
# baseline (speedup 1.0000x reference)
"""Contrastive-loss kernel for Trainium2 (8 NeuronCores, Bass/Tile).

Math: for sim = logits_flat @ labels_flat.T (N x N, N = 8192),
  loss = mean_i sum_j [ad_i == ad_j] * (-log2(clip(softmax(sim)_ij, 1e-12)))

Decomposition (pad_mask is all-ones for this problem):
  -log2(clip(p_ij, EPS)) = C - k*relu(sim_ij - (LSE_i - C*ln2)),  C = -log2(EPS)
  loss = (C*P - k * sum_{(i,j): ad_i==ad_j} relu(sim_ij - thr_i)) / N
with P = total positive-pair count (host-side, from ad_idxs alone) and
thr_i = LSE_i - C*ln2.

Rows are sorted by ad value on the host, so the positive pairs of any 128-row
tile live in a static 256-wide column window around the diagonal; window
labels + a gate tensor are shipped per-core as data (SPMD-identical program).

The scale s = 128/ln2 is folded into the bf16 operands (q *= sqrt(s),
l *= sqrt(s)) so PSUM holds x = s*sim; note k/s == 1/128 exactly.

The 8192-col LSE exp row-sums are split between two engines:
  - ACT chunks: native exp via activation(Exp, scale=1/s, bias=-64) with
    accum_out row sums (one instr per [128,2048] PSUM chunk).
  - DVE chunks: fast exp2 bit trick. bits = max(x + 16256, 0) truncated to
    uint16 equals the bf16 bit pattern of ~e^sim (d bits / d sim = s/ln2
    ... exactly 128*log2 e = s); a second tensor_scalar over the bitcast
    bf16 view with accum_out (op1=add) yields the row sum. The accum pass
    of each DVE chunk is deferred until after the NEXT DVE chunk's
    PSUM-draining pass, so PSUM slots free early and the PE never waits
    behind an SBUF-only op.
Both are combined as S = S_act + e^-64 * S_dve, LSE = ln(S) + 64.
A manual InstLoadActFuncSet of the exp+ln table at program start avoids the
mid-kernel table reload (1.3 us) before Ln.

Band epilogue is one fused DVE op per row tile, interleaved into the dense
phase per tile-pair so the final tail is short:
  scalar_tensor_tensor(out, band, -s*thr_i, gate, add, max, accum_out)
with gate = 0 for positive pairs and 2^20 for non-pairs; max(y, 2^20) = 2^20
exactly (|y| <= ~7000), so the host subtracts n_nonpair*2^20 per row.
Final: loss = (C*P - (1/128) * S_band_scaled) / N.
"""

import math
import sys

import numpy as np

sys.path.insert(0, "/opt/trn_rl_repo")

B, S, D = 8, 1024, 128
N = B * S  # 8192
NCORES = 8
ROWS_PER_CORE = N // NCORES  # 1024
TILES_PER_CORE = ROWS_PER_CORE // 128  # 8
NTILES = N // 128  # 64
CH = 2048  # dense chunk width (4 PSUM banks)
NCH = N // CH  # 4
MM_N = 512  # output free dim per matmul (PSUM one-bank limit)
MAXW = 512  # widest supported band window

EPS = 1e-12
C_BITS = -math.log2(EPS)  # 39.863137...
C_NATS = -math.log(EPS)  # 27.631021...
K_LOG2E = 1.0 / math.log(2.0)  # 1.442695...
SCALE = 128.0 / math.log(2.0)  # 184.6650...; k/SCALE == 1/128 exactly
SQRT_SCALE = math.sqrt(SCALE)
SHIFT = 64.0
GATE = float(2 ** 20)
EXP_BIAS = 16256.0  # 127 << 7: bf16 exponent bias in bit-space

# Global chunk assignment (index g = row_tile*4 + chunk): chunks in DVE_SET
# take the DVE fast-exp path, the rest the ACT native-exp path. 22/10 split
# (ACT ~2.3us vs DVE ~4.6us per chunk), no two DVE chunks adjacent, none in
# the last row tile (so its epilogue isn't gated on a slow DVE chain).
DVE_SET = frozenset({1, 4, 7, 10, 13, 16, 19, 22, 25, 27})

_programs = {}


def _build_program(W: int):
    """Build + compile the per-core Bass program for band width W."""
    import concourse.bass as bass
    from concourse import bacc, mybir, tile
    from concourse.hw_specs import get_activation_tables

    f32 = mybir.dt.float32
    bf16 = mybir.dt.bfloat16
    u16 = mybir.dt.uint16
    AF = mybir.ActivationFunctionType
    ALU = mybir.AluOpType
    NW = TILES_PER_CORE * W

    nc = bacc.Bacc("TRN2", target_bir_lowering=False, debug=False,
                   num_devices=NCORES)
    qt_d = nc.dram_tensor("qt", [128, ROWS_PER_CORE], bf16, kind="ExternalInput").ap()
    lt_d = nc.dram_tensor("lt", [128, N], bf16, kind="ExternalInput").ap()
    lw_d = nc.dram_tensor("lw", [128, NW], bf16, kind="ExternalInput").ap()
    zg_d = nc.dram_tensor("zg", [128, NW], bf16, kind="ExternalInput").ap()
    out_d = nc.dram_tensor("out", [128, TILES_PER_CORE], f32,
                           kind="ExternalOutput").ap()

    # Index of the one table set holding exp AND ln (plus relu/copy):
    # preloading it up front (under the initial DMAs) makes every later
    # activation table-hit, saving the 1.3us mid-kernel reload before Ln.
    table_names = list(get_activation_tables(nc.m.arch).keys())
    combined_id = table_names.index("natural_log_exp_and_others")

    with tile.TileContext(nc) as tc:
        with (
            tc.tile_pool(name="const", bufs=1) as constp,
            tc.tile_pool(name="psum", bufs=2, space=bass.MemorySpace.PSUM) as psump,
            tc.tile_pool(name="es", bufs=3) as esp,
            tc.tile_pool(name="bits", bufs=3) as bitsp,
            tc.tile_pool(name="small", bufs=2) as smallp,
        ):
            nc.scalar.add_instruction(
                mybir.InstLoadActFuncSet(
                    name=nc.get_next_instruction_name(),
                    ins=[], outs=[], act_func_set_id=combined_id,
                )
            )
            # Spread DMA triggers across the engine queues that may issue
            # them (SP/ACT/GpSimd): each trigger costs ~0.7us of serial
            # issue time on its queue.
            # DMA priority order: qt + lt0 first (dense tile 0 is emitted
            # before the band block), then lw for the bands, then the rest.
            qt = constp.tile([128, ROWS_PER_CORE], bf16, tag="qt")
            nc.sync.dma_start(qt[:], qt_d[:])
            lts = []
            for c in range(NCH):
                t = constp.tile([128, CH], bf16, tag=f"lt{c}")
                lts.append(t)

            def dma_lt(c):
                # split each chunk DMA so the first matmuls start sooner
                q = CH // 4
                for i in range(4):
                    nc.sync.dma_start(lts[c][:, i * q:(i + 1) * q],
                                      lt_d[:, c * CH + i * q:c * CH + (i + 1) * q])

            dma_lt(0)
            lw = constp.tile([128, NW], bf16, tag="lw")
            nc.sync.dma_start(lw[:], lw_d[:])
            for c in range(1, NCH):
                dma_lt(c)
            zg = constp.tile([128, NW], bf16, tag="zg")
            nc.sync.dma_start(zg[:], zg_d[:])

            shiftb = constp.tile([128, 1], f32, tag="shiftb")
            nc.vector.memset(shiftb[:], -SHIFT)
            bandsall = constp.tile([128, NW], bf16, tag="bandsall")
            # Per-(row tile, chunk) partial row sums for both engine paths.
            separts = constp.tile([128, TILES_PER_CORE, NCH], f32, tag="separts")
            sepd = constp.tile([128, TILES_PER_CORE, NCH], f32, tag="sepd")
            nc.vector.memset(separts[:], 0.0)
            nc.vector.memset(sepd[:], 0.0)
            bsum = constp.tile([128, TILES_PER_CORE], f32, tag="bsum")

            # Dense phase: bf16 matmuls -> PSUM [128,2048] chunks; ACT chunks
            # do native exp with accum row sums, DVE chunks the uint16 exp2
            # bit trick + bitcast tensor_scalar accum row sums.
            pending = []

            def flush_pending():
                while pending:
                    bb, r0, c0 = pending.pop(0)
                    nc.vector.tensor_scalar(bb, bb, 0.0, None,
                                            ALU.add, ALU.add,
                                            accum_out=sepd[:, r0, c0:c0 + 1])

            sesA = smallp.tile([128, TILES_PER_CORE], f32, tag="sesA")
            sesD = smallp.tile([128, TILES_PER_CORE], f32, tag="sesD")
            stot = smallp.tile([128, TILES_PER_CORE], f32, tag="stot")
            lse = smallp.tile([128, TILES_PER_CORE], f32, tag="lse")
            negthr = smallp.tile([128, TILES_PER_CORE], f32, tag="negthr")
            junk = smallp.tile([128, W], bf16, tag="junk")

            def pair_epilogue(r1):
                """Finish LSE + band loss for row tiles (r1-1, r1), inline so
                the tail after the last chunk is just one tile-pair's worth."""
                r0 = r1 - 1
                sl = slice(r0, r1 + 1)
                nc.vector.reduce_sum(sesA[:, sl], separts[:, sl, :],
                                     axis=mybir.AxisListType.X)
                nc.vector.reduce_sum(sesD[:, sl], sepd[:, sl, :],
                                     axis=mybir.AxisListType.X)
                nc.vector.scalar_tensor_tensor(stot[:, sl], sesD[:, sl],
                                               math.exp(-SHIFT), sesA[:, sl],
                                               ALU.mult, ALU.add)
                nc.scalar.activation(lse[:, sl], stot[:, sl], AF.Ln)
                # negthr = -SCALE*(lse + SHIFT - C_NATS)
                nc.vector.tensor_scalar(negthr[:, sl], lse[:, sl], -SCALE,
                                        -SCALE * (SHIFT - C_NATS),
                                        ALU.mult, ALU.add)
                for r in (r0, r1):
                    nc.vector.scalar_tensor_tensor(
                        junk[:], bandsall[:, r * W:(r + 1) * W],
                        negthr[:, r:r + 1], zg[:, r * W:(r + 1) * W],
                        ALU.add, ALU.max, accum_out=bsum[:, r:r + 1])

            def dense_tile(r, cs=range(NCH)):
                qtr = qt[:, r * 128:(r + 1) * 128]
                for c in cs:
                    ps = psump.tile([128, CH], f32, tag="ps")
                    for m in range(CH // MM_N):
                        nc.tensor.matmul(
                            ps[:, m * MM_N:(m + 1) * MM_N],
                            qtr,
                            lts[c][:, m * MM_N:(m + 1) * MM_N],
                        )
                    if (r * NCH + c) not in DVE_SET:
                        # in-place exp over the PSUM chunk: the elementwise
                        # result is discarded (only accum_out matters), and
                        # writing PSUM instead of SBUF keeps the SBUF write
                        # port free for the DVE's accum passes.
                        nc.scalar.activation(ps[:], ps[:], AF.Exp,
                                             bias=shiftb[:], scale=1.0 / SCALE,
                                             accum_out=separts[:, r, c:c + 1])
                    else:
                        bits = bitsp.tile([128, CH], u16, tag="bits")
                        nc.vector.tensor_scalar(bits[:], ps[:], EXP_BIAS, 0.0,
                                                ALU.add, ALU.max)
                        flush_pending()
                        pending.append((bits[:].bitcast(bf16), r, c))

            # Start order matched to DMA arrival: chunk (0,0) needs only
            # qt + lt0; the bands need qt + lw; chunks (0,1..3) need the
            # later lt chunks. The band block covers the lt1-3 DMA window.
            dense_tile(0, cs=[0])
            for r in range(TILES_PER_CORE):
                qtr = qt[:, r * 128:(r + 1) * 128]
                psb = psump.tile([128, W], f32, tag="ps")
                for m in range(0, W, MM_N):
                    w = min(MM_N, W - m)
                    nc.tensor.matmul(psb[:, m:m + w], qtr,
                                     lw[:, r * W + m:r * W + m + w])
                nc.vector.tensor_copy(bandsall[:, r * W:(r + 1) * W], psb[:])
            dense_tile(0, cs=[1, 2, 3])
            for r in range(1, TILES_PER_CORE):
                dense_tile(r)
                if r % 2 == 1:
                    flush_pending()
                    pair_epilogue(r)

            nc.sync.dma_start(out_d[:], bsum[:])

    nc.compile()
    return nc


def _get_program(W: int):
    if W not in _programs:
        _programs[W] = _build_program(W)
    return _programs[W]


def _host_reference(logits_flat, labels_flat, valid, ad):
    """Numpy fallback mirroring the reference exactly (pathological inputs)."""
    sim = logits_flat.astype(np.float64) @ labels_flat.astype(np.float64).T
    pv = valid[:, None] & valid[None, :]
    sim = np.where(pv, sim, -np.inf)
    m = np.max(sim, axis=-1, keepdims=True)
    e = np.exp(sim - m)
    p = e / np.sum(e, axis=-1, keepdims=True)
    lm = ((ad[:, None] == ad[None, :]) & pv).astype(np.float64)
    pl = -np.log2(np.clip(p, EPS, None)) * lm
    return np.float32(pl.sum(axis=-1).mean())


def _prepare(logits_flat, labels_flat, ad):
    order = np.argsort(ad, kind="stable")
    ads = ad[order]
    Q = logits_flat[order]
    L = labels_flat[order]

    change = np.empty(N, dtype=bool)
    change[0] = True
    change[1:] = ads[1:] != ads[:-1]
    run_id = np.cumsum(change) - 1
    run_start = np.flatnonzero(change)
    run_len = np.diff(np.append(run_start, N))
    row_start = run_start[run_id]  # group start per (sorted) row
    row_end = row_start + run_len[run_id]
    p_total = int(np.sum(run_len.astype(np.int64) ** 2))

    tile_of_row = np.arange(N) // 128
    W = 256
    A = None
    while W <= MAXW:
        A = np.clip(np.arange(NTILES) * 128 - (W - 128) // 2, 0, N - W)
        if np.all((row_start >= A[tile_of_row]) & (row_end <= A[tile_of_row] + W)):
            break
        W *= 2
    else:
        return None  # pathological ad distribution; caller falls back
    return order, ads, Q, L, p_total, W, A


def _make_in_maps(Q, L, ads, A, W):
    import ml_dtypes

    LT = np.ascontiguousarray((L * SQRT_SCALE).T)  # [128, N] f32
    LTb = LT.astype(ml_dtypes.bfloat16)
    in_maps = []
    for d in range(NCORES):
        rows = slice(d * ROWS_PER_CORE, (d + 1) * ROWS_PER_CORE)
        qt_np = np.ascontiguousarray(
            (Q[rows] * SQRT_SCALE).T.astype(ml_dtypes.bfloat16))
        lw_np = np.empty((128, TILES_PER_CORE * W), dtype=ml_dtypes.bfloat16)
        zg_np = np.empty((128, TILES_PER_CORE * W), dtype=ml_dtypes.bfloat16)
        for r in range(TILES_PER_CORE):
            g = d * TILES_PER_CORE + r
            a = int(A[g])
            lw_np[:, r * W:(r + 1) * W] = LTb[:, a:a + W]
            eq = ads[a:a + W][None, :] == ads[g * 128:(g + 1) * 128][:, None]
            zg_np[:, r * W:(r + 1) * W] = np.where(eq, 0.0, GATE)
        in_maps.append({"qt": qt_np, "lt": LTb, "lw": lw_np, "zg": zg_np})
    return in_maps


def _make_corrections(ads, A, W):
    """Per-core [128, TILES] non-pair counts for the gate offset."""
    corr = []
    for d in range(NCORES):
        c = np.empty((128, TILES_PER_CORE), dtype=np.int64)
        for r in range(TILES_PER_CORE):
            g = d * TILES_PER_CORE + r
            a = int(A[g])
            eq = ads[a:a + W][None, :] == ads[g * 128:(g + 1) * 128][:, None]
            c[:, r] = W - eq.sum(axis=1)
        corr.append(c)
    return corr


def kernel(logits, labels, pad_mask, ad_idxs):
    logits_flat = np.ascontiguousarray(
        np.asarray(logits, dtype=np.float32).reshape(N, D))
    labels_flat = np.ascontiguousarray(
        np.asarray(labels, dtype=np.float32).reshape(N, D))
    valid = np.asarray(pad_mask).reshape(N) != 0
    ad = np.asarray(ad_idxs).reshape(N).astype(np.int64)

    if not valid.all():
        return _host_reference(logits_flat, labels_flat, valid, ad)

    prep = _prepare(logits_flat, labels_flat, ad)
    if prep is None:
        return _host_reference(logits_flat, labels_flat, valid, ad)
    order, ads, Q, L, p_total, W, A = prep

    nc = _get_program(W)
    in_maps = _make_in_maps(Q, L, ads, A, W)
    corr = _make_corrections(ads, A, W)

    from concourse import bass_utils
    res = bass_utils.run_bass_kernel_spmd(nc, in_maps, core_ids=list(range(NCORES)))
    s_scaled = 0.0
    for d, r in enumerate(res.results):
        bs = np.asarray(r["out"], dtype=np.float64)
        s_scaled += float(bs.sum()) - float(corr[d].sum()) * GATE
    loss = (C_BITS * p_total - s_scaled / 128.0) / N
    return np.float32(loss)


# revision 38
# speedup vs baseline: 1.0038x; 1.0038x over previous
"""Contrastive-loss kernel for Trainium2 (8 NeuronCores, Bass/Tile).

Math: for sim = logits_flat @ labels_flat.T (N x N, N = 8192),
  loss = mean_i sum_j [ad_i == ad_j] * (-log2(clip(softmax(sim)_ij, 1e-12)))

Decomposition (pad_mask is all-ones for this problem):
  -log2(clip(p_ij, EPS)) = C - k*relu(sim_ij - (LSE_i - C*ln2)),  C = -log2(EPS)
  loss = (C*P - k * sum_{(i,j): ad_i==ad_j} relu(sim_ij - thr_i)) / N
with P = total positive-pair count (host-side, from ad_idxs alone) and
thr_i = LSE_i - C*ln2.

Rows are sorted by ad value on the host, so the positive pairs of any 128-row
tile live in a static 256-wide column window around the diagonal; window
labels + a gate tensor are shipped per-core as data (SPMD-identical program).

The scale s = 128/ln2 is folded into the bf16 operands (q *= sqrt(s),
l *= sqrt(s)) so PSUM holds x = s*sim; note k/s == 1/128 exactly.

The 8192-col LSE exp row-sums are split between two engines:
  - ACT chunks: native exp via activation(Exp, scale=1/s, bias=-64) with
    accum_out row sums (one instr per [128,2048] PSUM chunk).
  - DVE chunks: fast exp2 bit trick. bits = max(x + 16256, 0) truncated to
    uint16 equals the bf16 bit pattern of ~e^sim (d bits / d sim = s/ln2
    ... exactly 128*log2 e = s); a second tensor_scalar over the bitcast
    bf16 view with accum_out (op1=add) yields the row sum. The accum pass
    of each DVE chunk is deferred until after the NEXT DVE chunk's
    PSUM-draining pass, so PSUM slots free early and the PE never waits
    behind an SBUF-only op.
Both are combined as S = S_act + e^-64 * S_dve, LSE = ln(S) + 64.
A manual InstLoadActFuncSet of the exp+ln table at program start avoids the
mid-kernel table reload (1.3 us) before Ln.

Band epilogue is one fused DVE op per row tile, interleaved into the dense
phase per tile-pair so the final tail is short:
  scalar_tensor_tensor(out, band, -s*thr_i, gate, add, max, accum_out)
with gate = 0 for positive pairs and 2^20 for non-pairs; max(y, 2^20) = 2^20
exactly (|y| <= ~7000), so the host subtracts n_nonpair*2^20 per row.
Final: loss = (C*P - (1/128) * S_band_scaled) / N.
"""

import math
import sys

import numpy as np

sys.path.insert(0, "/opt/trn_rl_repo")

B, S, D = 8, 1024, 128
N = B * S  # 8192
NCORES = 8
ROWS_PER_CORE = N // NCORES  # 1024
TILES_PER_CORE = ROWS_PER_CORE // 128  # 8
NTILES = N // 128  # 64
CH = 2048  # dense chunk width (4 PSUM banks)
NCH = N // CH  # 4
MM_N = 512  # output free dim per matmul (PSUM one-bank limit)
MAXW = 512  # widest supported band window

EPS = 1e-12
C_BITS = -math.log2(EPS)  # 39.863137...
C_NATS = -math.log(EPS)  # 27.631021...
K_LOG2E = 1.0 / math.log(2.0)  # 1.442695...
SCALE = 128.0 / math.log(2.0)  # 184.6650...; k/SCALE == 1/128 exactly
SQRT_SCALE = math.sqrt(SCALE)
SHIFT = 64.0
GATE = float(2 ** 20)
EXP_BIAS = 16256.0  # 127 << 7: bf16 exponent bias in bit-space

# Global chunk assignment (index g = row_tile*4 + chunk): chunks in DVE_SET
# take the DVE fast-exp path, the rest the ACT native-exp path. 22/10 split
# (ACT ~2.3us vs DVE ~4.6us per chunk), no two DVE chunks adjacent, none in
# the last row tile (so its epilogue isn't gated on a slow DVE chain).
DVE_SET = frozenset({1, 4, 7, 10, 13, 16, 19, 22, 25, 27})

_programs = {}


def _build_program(W: int):
    """Build + compile the per-core Bass program for band width W."""
    import concourse.bass as bass
    from concourse import bacc, mybir, tile
    from concourse.hw_specs import get_activation_tables

    f32 = mybir.dt.float32
    bf16 = mybir.dt.bfloat16
    u16 = mybir.dt.uint16
    AF = mybir.ActivationFunctionType
    ALU = mybir.AluOpType
    NW = TILES_PER_CORE * W

    nc = bacc.Bacc("TRN2", target_bir_lowering=False, debug=False,
                   num_devices=NCORES)
    qt_d = nc.dram_tensor("qt", [128, ROWS_PER_CORE], bf16, kind="ExternalInput").ap()
    lt_d = nc.dram_tensor("lt", [128, N], bf16, kind="ExternalInput").ap()
    lw_d = nc.dram_tensor("lw", [128, NW], bf16, kind="ExternalInput").ap()
    zg_d = nc.dram_tensor("zg", [128, NW], bf16, kind="ExternalInput").ap()
    out_d = nc.dram_tensor("out", [128, TILES_PER_CORE], f32,
                           kind="ExternalOutput").ap()

    # Index of the one table set holding exp AND ln (plus relu/copy):
    # preloading it up front (under the initial DMAs) makes every later
    # activation table-hit, saving the 1.3us mid-kernel reload before Ln.
    table_names = list(get_activation_tables(nc.m.arch).keys())
    combined_id = table_names.index("natural_log_exp_and_others")

    with tile.TileContext(nc) as tc:
        with (
            tc.tile_pool(name="const", bufs=1) as constp,
            tc.tile_pool(name="psum", bufs=2, space=bass.MemorySpace.PSUM) as psump,
            tc.tile_pool(name="es", bufs=3) as esp,
            tc.tile_pool(name="bits", bufs=3) as bitsp,
            tc.tile_pool(name="small", bufs=2) as smallp,
        ):
            nc.scalar.add_instruction(
                mybir.InstLoadActFuncSet(
                    name=nc.get_next_instruction_name(),
                    ins=[], outs=[], act_func_set_id=combined_id,
                )
            )
            # Spread DMA triggers across the engine queues that may issue
            # them (SP/ACT/GpSimd): each trigger costs ~0.7us of serial
            # issue time on its queue.
            # DMA priority order: qt + lt0 first (dense tile 0 is emitted
            # before the band block), then lw for the bands, then the rest.
            qt = constp.tile([128, ROWS_PER_CORE], bf16, tag="qt")
            nc.sync.dma_start(qt[:], qt_d[:])
            lts = []
            for c in range(NCH):
                t = constp.tile([128, CH], bf16, tag=f"lt{c}")
                lts.append(t)

            def dma_lt(c):
                # split each chunk DMA so the first matmuls start sooner
                q = CH // 4
                for i in range(4):
                    nc.sync.dma_start(lts[c][:, i * q:(i + 1) * q],
                                      lt_d[:, c * CH + i * q:c * CH + (i + 1) * q])

            dma_lt(0)
            lw = constp.tile([128, NW], bf16, tag="lw")
            nc.sync.dma_start(lw[:], lw_d[:])
            for c in range(1, NCH):
                dma_lt(c)
            zg = constp.tile([128, NW], bf16, tag="zg")
            nc.sync.dma_start(zg[:], zg_d[:])

            shiftb = constp.tile([128, 1], f32, tag="shiftb")
            nc.vector.memset(shiftb[:], -SHIFT)
            bandsall = constp.tile([128, NW], bf16, tag="bandsall")
            # Per-(row tile, chunk) partial row sums for both engine paths.
            separts = constp.tile([128, TILES_PER_CORE, NCH], f32, tag="separts")
            sepd = constp.tile([128, TILES_PER_CORE, NCH], f32, tag="sepd")
            nc.vector.memset(separts[:], 0.0)
            nc.vector.memset(sepd[:], 0.0)
            bsum = constp.tile([128, TILES_PER_CORE], f32, tag="bsum")

            # Dense phase: bf16 matmuls -> PSUM [128,2048] chunks; ACT chunks
            # do native exp with accum row sums, DVE chunks the uint16 exp2
            # bit trick + bitcast tensor_scalar accum row sums.
            pending = []

            def flush_pending():
                while pending:
                    bb, r0, c0 = pending.pop(0)
                    nc.vector.tensor_scalar(bb, bb, 0.0, None,
                                            ALU.add, ALU.add,
                                            accum_out=sepd[:, r0, c0:c0 + 1])

            sesA = smallp.tile([128, TILES_PER_CORE], f32, tag="sesA")
            sesD = smallp.tile([128, TILES_PER_CORE], f32, tag="sesD")
            stot = smallp.tile([128, TILES_PER_CORE], f32, tag="stot")
            lse = smallp.tile([128, TILES_PER_CORE], f32, tag="lse")
            negthr = smallp.tile([128, TILES_PER_CORE], f32, tag="negthr")
            junk = smallp.tile([128, W], bf16, tag="junk")

            def pair_epilogue(r1):
                """Finish LSE + band loss for row tiles (r1-1, r1), inline so
                the tail after the last chunk is just one tile-pair's worth."""
                r0 = r1 - 1
                sl = slice(r0, r1 + 1)
                nc.vector.reduce_sum(sesA[:, sl], separts[:, sl, :],
                                     axis=mybir.AxisListType.X)
                nc.vector.reduce_sum(sesD[:, sl], sepd[:, sl, :],
                                     axis=mybir.AxisListType.X)
                nc.vector.scalar_tensor_tensor(stot[:, sl], sesD[:, sl],
                                               math.exp(-SHIFT), sesA[:, sl],
                                               ALU.mult, ALU.add)
                nc.scalar.activation(lse[:, sl], stot[:, sl], AF.Ln)
                # negthr = -SCALE*(lse + SHIFT - C_NATS)
                nc.vector.tensor_scalar(negthr[:, sl], lse[:, sl], -SCALE,
                                        -SCALE * (SHIFT - C_NATS),
                                        ALU.mult, ALU.add)
                for r in (r0, r1):
                    nc.vector.scalar_tensor_tensor(
                        junk[:], bandsall[:, r * W:(r + 1) * W],
                        negthr[:, r:r + 1], zg[:, r * W:(r + 1) * W],
                        ALU.add, ALU.max, accum_out=bsum[:, r:r + 1])

            def dense_tile(r, cs=range(NCH)):
                qtr = qt[:, r * 128:(r + 1) * 128]
                for c in cs:
                    ps = psump.tile([128, CH], f32, tag="ps")
                    for m in range(CH // MM_N):
                        nc.tensor.matmul(
                            ps[:, m * MM_N:(m + 1) * MM_N],
                            qtr,
                            lts[c][:, m * MM_N:(m + 1) * MM_N],
                        )
                    if (r * NCH + c) not in DVE_SET:
                        # the elementwise result is discarded (only accum_out
                        # matters) — uint8 out halves the SBUF write traffic
                        es = esp.tile([128, CH], mybir.dt.uint8, tag="es")
                        nc.scalar.activation(es[:], ps[:], AF.Exp,
                                             bias=shiftb[:], scale=1.0 / SCALE,
                                             accum_out=separts[:, r, c:c + 1])
                    else:
                        bits = bitsp.tile([128, CH], u16, tag="bits")
                        nc.vector.tensor_scalar(bits[:], ps[:], EXP_BIAS, 0.0,
                                                ALU.add, ALU.max)
                        flush_pending()
                        pending.append((bits[:].bitcast(bf16), r, c))

            # Start order matched to DMA arrival: chunk (0,0) needs only
            # qt + lt0; the bands need qt + lw; chunks (0,1..3) need the
            # later lt chunks. The band block covers the lt1-3 DMA window.
            dense_tile(0, cs=[0])
            for r in range(TILES_PER_CORE):
                qtr = qt[:, r * 128:(r + 1) * 128]
                psb = psump.tile([128, W], f32, tag="ps")
                for m in range(0, W, MM_N):
                    w = min(MM_N, W - m)
                    nc.tensor.matmul(psb[:, m:m + w], qtr,
                                     lw[:, r * W + m:r * W + m + w])
                nc.vector.tensor_copy(bandsall[:, r * W:(r + 1) * W], psb[:])
            dense_tile(0, cs=[1, 2, 3])
            for r in range(1, TILES_PER_CORE):
                dense_tile(r)
                if r % 2 == 1:
                    flush_pending()
                    pair_epilogue(r)

            nc.sync.dma_start(out_d[:], bsum[:])

    nc.compile()
    return nc


def _get_program(W: int):
    if W not in _programs:
        _programs[W] = _build_program(W)
    return _programs[W]


def _host_reference(logits_flat, labels_flat, valid, ad):
    """Numpy fallback mirroring the reference exactly (pathological inputs)."""
    sim = logits_flat.astype(np.float64) @ labels_flat.astype(np.float64).T
    pv = valid[:, None] & valid[None, :]
    sim = np.where(pv, sim, -np.inf)
    m = np.max(sim, axis=-1, keepdims=True)
    e = np.exp(sim - m)
    p = e / np.sum(e, axis=-1, keepdims=True)
    lm = ((ad[:, None] == ad[None, :]) & pv).astype(np.float64)
    pl = -np.log2(np.clip(p, EPS, None)) * lm
    return np.float32(pl.sum(axis=-1).mean())


def _prepare(logits_flat, labels_flat, ad):
    order = np.argsort(ad, kind="stable")
    ads = ad[order]
    Q = logits_flat[order]
    L = labels_flat[order]

    change = np.empty(N, dtype=bool)
    change[0] = True
    change[1:] = ads[1:] != ads[:-1]
    run_id = np.cumsum(change) - 1
    run_start = np.flatnonzero(change)
    run_len = np.diff(np.append(run_start, N))
    row_start = run_start[run_id]  # group start per (sorted) row
    row_end = row_start + run_len[run_id]
    p_total = int(np.sum(run_len.astype(np.int64) ** 2))

    tile_of_row = np.arange(N) // 128
    W = 256
    A = None
    while W <= MAXW:
        A = np.clip(np.arange(NTILES) * 128 - (W - 128) // 2, 0, N - W)
        if np.all((row_start >= A[tile_of_row]) & (row_end <= A[tile_of_row] + W)):
            break
        W *= 2
    else:
        return None  # pathological ad distribution; caller falls back
    return order, ads, Q, L, p_total, W, A


def _make_in_maps(Q, L, ads, A, W):
    import ml_dtypes

    LT = np.ascontiguousarray((L * SQRT_SCALE).T)  # [128, N] f32
    LTb = LT.astype(ml_dtypes.bfloat16)
    in_maps = []
    for d in range(NCORES):
        rows = slice(d * ROWS_PER_CORE, (d + 1) * ROWS_PER_CORE)
        qt_np = np.ascontiguousarray(
            (Q[rows] * SQRT_SCALE).T.astype(ml_dtypes.bfloat16))
        lw_np = np.empty((128, TILES_PER_CORE * W), dtype=ml_dtypes.bfloat16)
        zg_np = np.empty((128, TILES_PER_CORE * W), dtype=ml_dtypes.bfloat16)
        for r in range(TILES_PER_CORE):
            g = d * TILES_PER_CORE + r
            a = int(A[g])
            lw_np[:, r * W:(r + 1) * W] = LTb[:, a:a + W]
            eq = ads[a:a + W][None, :] == ads[g * 128:(g + 1) * 128][:, None]
            zg_np[:, r * W:(r + 1) * W] = np.where(eq, 0.0, GATE)
        in_maps.append({"qt": qt_np, "lt": LTb, "lw": lw_np, "zg": zg_np})
    return in_maps


def _make_corrections(ads, A, W):
    """Per-core [128, TILES] non-pair counts for the gate offset."""
    corr = []
    for d in range(NCORES):
        c = np.empty((128, TILES_PER_CORE), dtype=np.int64)
        for r in range(TILES_PER_CORE):
            g = d * TILES_PER_CORE + r
            a = int(A[g])
            eq = ads[a:a + W][None, :] == ads[g * 128:(g + 1) * 128][:, None]
            c[:, r] = W - eq.sum(axis=1)
        corr.append(c)
    return corr


def kernel(logits, labels, pad_mask, ad_idxs):
    logits_flat = np.ascontiguousarray(
        np.asarray(logits, dtype=np.float32).reshape(N, D))
    labels_flat = np.ascontiguousarray(
        np.asarray(labels, dtype=np.float32).reshape(N, D))
    valid = np.asarray(pad_mask).reshape(N) != 0
    ad = np.asarray(ad_idxs).reshape(N).astype(np.int64)

    if not valid.all():
        return _host_reference(logits_flat, labels_flat, valid, ad)

    prep = _prepare(logits_flat, labels_flat, ad)
    if prep is None:
        return _host_reference(logits_flat, labels_flat, valid, ad)
    order, ads, Q, L, p_total, W, A = prep

    nc = _get_program(W)
    in_maps = _make_in_maps(Q, L, ads, A, W)
    corr = _make_corrections(ads, A, W)

    from concourse import bass_utils
    res = bass_utils.run_bass_kernel_spmd(nc, in_maps, core_ids=list(range(NCORES)))
    s_scaled = 0.0
    for d, r in enumerate(res.results):
        bs = np.asarray(r["out"], dtype=np.float64)
        s_scaled += float(bs.sum()) - float(corr[d].sum()) * GATE
    loss = (C_BITS * p_total - s_scaled / 128.0) / N
    return np.float32(loss)


# revision 39
# speedup vs baseline: 1.0389x; 1.0349x over previous
"""Contrastive-loss kernel for Trainium2 (8 NeuronCores, Bass/Tile).

Math: for sim = logits_flat @ labels_flat.T (N x N, N = 8192),
  loss = mean_i sum_j [ad_i == ad_j] * (-log2(clip(softmax(sim)_ij, 1e-12)))

Decomposition (pad_mask is all-ones for this problem):
  -log2(clip(p_ij, EPS)) = C - k*relu(sim_ij - (LSE_i - C*ln2)),  C = -log2(EPS)
  loss = (C*P - k * sum_{(i,j): ad_i==ad_j} relu(sim_ij - thr_i)) / N
with P = total positive-pair count (host-side, from ad_idxs alone) and
thr_i = LSE_i - C*ln2.

Rows are sorted by ad value on the host, so the positive pairs of any 128-row
tile live in a static 256-wide column window around the diagonal; window
labels + a gate tensor are shipped per-core as data (SPMD-identical program).

The scale s = 128/ln2 is folded into the bf16 operands (q *= sqrt(s),
l *= sqrt(s)) so PSUM holds x = s*sim; note k/s == 1/128 exactly.

The 8192-col LSE exp row-sums are split between two engines:
  - ACT chunks: native exp via activation(Exp, scale=1/s, bias=-64) with
    accum_out row sums (one instr per [128,2048] PSUM chunk).
  - DVE chunks: fast exp2 bit trick. bits = max(x + 16256, 0) truncated to
    uint16 equals the bf16 bit pattern of ~e^sim (d bits / d sim = s/ln2
    ... exactly 128*log2 e = s); a second tensor_scalar over the bitcast
    bf16 view with accum_out (op1=add) yields the row sum. The accum pass
    of each DVE chunk is deferred until after the NEXT DVE chunk's
    PSUM-draining pass, so PSUM slots free early and the PE never waits
    behind an SBUF-only op.
Both are combined as S = S_act + e^-64 * S_dve, LSE = ln(S) + 64.
A manual InstLoadActFuncSet of the exp+ln table at program start avoids the
mid-kernel table reload (1.3 us) before Ln.

Band epilogue is one fused DVE op per row tile, interleaved into the dense
phase per tile-pair so the final tail is short:
  scalar_tensor_tensor(out, band, -s*thr_i, gate, add, max, accum_out)
with gate = 0 for positive pairs and 2^20 for non-pairs; max(y, 2^20) = 2^20
exactly (|y| <= ~7000), so the host subtracts n_nonpair*2^20 per row.
Final: loss = (C*P - (1/128) * S_band_scaled) / N.
"""

import math
import sys

import numpy as np

sys.path.insert(0, "/opt/trn_rl_repo")

B, S, D = 8, 1024, 128
N = B * S  # 8192
NCORES = 8
ROWS_PER_CORE = N // NCORES  # 1024
TILES_PER_CORE = ROWS_PER_CORE // 128  # 8
NTILES = N // 128  # 64
CH = 2048  # dense chunk width (4 PSUM banks)
NCH = N // CH  # 4
MM_N = 512  # output free dim per matmul (PSUM one-bank limit)
MAXW = 512  # widest supported band window

EPS = 1e-12
C_BITS = -math.log2(EPS)  # 39.863137...
C_NATS = -math.log(EPS)  # 27.631021...
K_LOG2E = 1.0 / math.log(2.0)  # 1.442695...
SCALE = 128.0 / math.log(2.0)  # 184.6650...; k/SCALE == 1/128 exactly
SQRT_SCALE = math.sqrt(SCALE)
SHIFT = 64.0
GATE = float(2 ** 20)
EXP_BIAS = 16256.0  # 127 << 7: bf16 exponent bias in bit-space

# Global chunk assignment (index g = row_tile*4 + chunk): chunks in DVE_SET
# take the DVE fast-exp path, the rest the ACT native-exp path. 22/10 split
# (ACT ~2.3us vs DVE ~4.6us per chunk), no two DVE chunks adjacent, none in
# the last row tile (so its epilogue isn't gated on a slow DVE chain).
DVE_SET = frozenset({1, 4, 7, 10, 13, 16, 19, 22, 25, 27})

_programs = {}


def _build_program(W: int):
    """Build + compile the per-core Bass program for band width W."""
    import concourse.bass as bass
    from concourse import bacc, mybir, tile
    from concourse.hw_specs import get_activation_tables

    f32 = mybir.dt.float32
    bf16 = mybir.dt.bfloat16
    u16 = mybir.dt.uint16
    AF = mybir.ActivationFunctionType
    ALU = mybir.AluOpType
    NW = TILES_PER_CORE * W

    nc = bacc.Bacc("TRN2", target_bir_lowering=False, debug=False,
                   num_devices=NCORES)
    f8 = mybir.dt.float8e4
    qt_d = nc.dram_tensor("qt", [128, ROWS_PER_CORE], f8, kind="ExternalInput").ap()
    lt_d = nc.dram_tensor("lt", [128, N], f8, kind="ExternalInput").ap()
    lw_d = nc.dram_tensor("lw", [128, NW], f8, kind="ExternalInput").ap()
    zg_d = nc.dram_tensor("zg", [128, NW], bf16, kind="ExternalInput").ap()
    out_d = nc.dram_tensor("out", [128, TILES_PER_CORE], f32,
                           kind="ExternalOutput").ap()

    # Index of the one table set holding exp AND ln (plus relu/copy):
    # preloading it up front (under the initial DMAs) makes every later
    # activation table-hit, saving the 1.3us mid-kernel reload before Ln.
    table_names = list(get_activation_tables(nc.m.arch).keys())
    combined_id = table_names.index("natural_log_exp_and_others")

    with tile.TileContext(nc) as tc:
        with (
            tc.tile_pool(name="const", bufs=1) as constp,
            tc.tile_pool(name="psum", bufs=2, space=bass.MemorySpace.PSUM) as psump,
            tc.tile_pool(name="es", bufs=3) as esp,
            tc.tile_pool(name="bits", bufs=3) as bitsp,
            tc.tile_pool(name="small", bufs=2) as smallp,
        ):
            nc.scalar.add_instruction(
                mybir.InstLoadActFuncSet(
                    name=nc.get_next_instruction_name(),
                    ins=[], outs=[], act_func_set_id=combined_id,
                )
            )
            # Spread DMA triggers across the engine queues that may issue
            # them (SP/ACT/GpSimd): each trigger costs ~0.7us of serial
            # issue time on its queue.
            # DMA priority order: qt + lt0 first (dense tile 0 is emitted
            # before the band block), then lw for the bands, then the rest.
            qt = constp.tile([128, ROWS_PER_CORE], f8, tag="qt")
            nc.sync.dma_start(qt[:], qt_d[:])
            lts = []
            for c in range(NCH):
                t = constp.tile([128, CH], f8, tag=f"lt{c}")
                lts.append(t)

            def dma_lt(c):
                # split each chunk DMA so the first matmuls start sooner
                q = CH // 4
                for i in range(4):
                    nc.sync.dma_start(lts[c][:, i * q:(i + 1) * q],
                                      lt_d[:, c * CH + i * q:c * CH + (i + 1) * q])

            dma_lt(0)
            lw = constp.tile([128, NW], f8, tag="lw")
            nc.sync.dma_start(lw[:], lw_d[:])
            for c in range(1, NCH):
                dma_lt(c)
            zg = constp.tile([128, NW], bf16, tag="zg")
            nc.sync.dma_start(zg[:], zg_d[:])

            shiftb = constp.tile([128, 1], f32, tag="shiftb")
            nc.vector.memset(shiftb[:], -SHIFT)
            bandsall = constp.tile([128, NW], bf16, tag="bandsall")
            # Per-(row tile, chunk) partial row sums for both engine paths.
            separts = constp.tile([128, TILES_PER_CORE, NCH], f32, tag="separts")
            sepd = constp.tile([128, TILES_PER_CORE, NCH], f32, tag="sepd")
            nc.vector.memset(separts[:], 0.0)
            nc.vector.memset(sepd[:], 0.0)
            bsum = constp.tile([128, TILES_PER_CORE], f32, tag="bsum")

            # Dense phase: bf16 matmuls -> PSUM [128,2048] chunks; ACT chunks
            # do native exp with accum row sums, DVE chunks the uint16 exp2
            # bit trick + bitcast tensor_scalar accum row sums.
            pending = []

            def flush_pending():
                while pending:
                    bb, r0, c0 = pending.pop(0)
                    nc.vector.tensor_scalar(bb, bb, 0.0, None,
                                            ALU.add, ALU.add,
                                            accum_out=sepd[:, r0, c0:c0 + 1])

            sesA = smallp.tile([128, TILES_PER_CORE], f32, tag="sesA")
            sesD = smallp.tile([128, TILES_PER_CORE], f32, tag="sesD")
            stot = smallp.tile([128, TILES_PER_CORE], f32, tag="stot")
            lse = smallp.tile([128, TILES_PER_CORE], f32, tag="lse")
            negthr = smallp.tile([128, TILES_PER_CORE], f32, tag="negthr")
            junk = smallp.tile([128, W], bf16, tag="junk")

            def pair_epilogue(r1):
                """Finish LSE + band loss for row tiles (r1-1, r1), inline so
                the tail after the last chunk is just one tile-pair's worth."""
                r0 = r1 - 1
                sl = slice(r0, r1 + 1)
                nc.vector.reduce_sum(sesA[:, sl], separts[:, sl, :],
                                     axis=mybir.AxisListType.X)
                nc.vector.reduce_sum(sesD[:, sl], sepd[:, sl, :],
                                     axis=mybir.AxisListType.X)
                nc.vector.scalar_tensor_tensor(stot[:, sl], sesD[:, sl],
                                               math.exp(-SHIFT), sesA[:, sl],
                                               ALU.mult, ALU.add)
                nc.scalar.activation(lse[:, sl], stot[:, sl], AF.Ln)
                # negthr = -SCALE*(lse + SHIFT - C_NATS)
                nc.vector.tensor_scalar(negthr[:, sl], lse[:, sl], -SCALE,
                                        -SCALE * (SHIFT - C_NATS),
                                        ALU.mult, ALU.add)
                for r in (r0, r1):
                    nc.vector.scalar_tensor_tensor(
                        junk[:], bandsall[:, r * W:(r + 1) * W],
                        negthr[:, r:r + 1], zg[:, r * W:(r + 1) * W],
                        ALU.add, ALU.max, accum_out=bsum[:, r:r + 1])

            def dense_tile(r, cs=range(NCH)):
                qtr = qt[:, r * 128:(r + 1) * 128]
                for c in cs:
                    ps = psump.tile([128, CH], f32, tag="ps")
                    for m in range(CH // MM_N):
                        nc.tensor.matmul(
                            ps[:, m * MM_N:(m + 1) * MM_N],
                            qtr,
                            lts[c][:, m * MM_N:(m + 1) * MM_N],
                        )
                    if (r * NCH + c) not in DVE_SET:
                        # the elementwise result is discarded (only accum_out
                        # matters) — uint8 out halves the SBUF write traffic
                        es = esp.tile([128, CH], mybir.dt.uint8, tag="es")
                        nc.scalar.activation(es[:], ps[:], AF.Exp,
                                             bias=shiftb[:], scale=1.0 / SCALE,
                                             accum_out=separts[:, r, c:c + 1])
                    else:
                        bits = bitsp.tile([128, CH], u16, tag="bits")
                        nc.vector.tensor_scalar(bits[:], ps[:], EXP_BIAS, 0.0,
                                                ALU.add, ALU.max)
                        flush_pending()
                        pending.append((bits[:].bitcast(bf16), r, c))

            # Start order matched to DMA arrival: chunk (0,0) needs only
            # qt + lt0; the bands need qt + lw; chunks (0,1..3) need the
            # later lt chunks. The band block covers the lt1-3 DMA window.
            dense_tile(0, cs=[0])
            for r in range(TILES_PER_CORE):
                qtr = qt[:, r * 128:(r + 1) * 128]
                psb = psump.tile([128, W], f32, tag="ps")
                for m in range(0, W, MM_N):
                    w = min(MM_N, W - m)
                    nc.tensor.matmul(psb[:, m:m + w], qtr,
                                     lw[:, r * W + m:r * W + m + w])
                nc.vector.tensor_copy(bandsall[:, r * W:(r + 1) * W], psb[:])
            dense_tile(0, cs=[1, 2, 3])
            for r in range(1, TILES_PER_CORE):
                dense_tile(r)
                if r % 2 == 1:
                    flush_pending()
                    pair_epilogue(r)

            nc.sync.dma_start(out_d[:], bsum[:])

    nc.compile()
    return nc


def _get_program(W: int):
    if W not in _programs:
        _programs[W] = _build_program(W)
    return _programs[W]


def _host_reference(logits_flat, labels_flat, valid, ad):
    """Numpy fallback mirroring the reference exactly (pathological inputs)."""
    sim = logits_flat.astype(np.float64) @ labels_flat.astype(np.float64).T
    pv = valid[:, None] & valid[None, :]
    sim = np.where(pv, sim, -np.inf)
    m = np.max(sim, axis=-1, keepdims=True)
    e = np.exp(sim - m)
    p = e / np.sum(e, axis=-1, keepdims=True)
    lm = ((ad[:, None] == ad[None, :]) & pv).astype(np.float64)
    pl = -np.log2(np.clip(p, EPS, None)) * lm
    return np.float32(pl.sum(axis=-1).mean())


def _prepare(logits_flat, labels_flat, ad):
    order = np.argsort(ad, kind="stable")
    ads = ad[order]
    Q = logits_flat[order]
    L = labels_flat[order]

    change = np.empty(N, dtype=bool)
    change[0] = True
    change[1:] = ads[1:] != ads[:-1]
    run_id = np.cumsum(change) - 1
    run_start = np.flatnonzero(change)
    run_len = np.diff(np.append(run_start, N))
    row_start = run_start[run_id]  # group start per (sorted) row
    row_end = row_start + run_len[run_id]
    p_total = int(np.sum(run_len.astype(np.int64) ** 2))

    tile_of_row = np.arange(N) // 128
    W = 256
    A = None
    while W <= MAXW:
        A = np.clip(np.arange(NTILES) * 128 - (W - 128) // 2, 0, N - W)
        if np.all((row_start >= A[tile_of_row]) & (row_end <= A[tile_of_row] + W)):
            break
        W *= 2
    else:
        return None  # pathological ad distribution; caller falls back
    return order, ads, Q, L, p_total, W, A


def _make_in_maps(Q, L, ads, A, W):
    import ml_dtypes

    LT = np.ascontiguousarray((L * SQRT_SCALE).T)  # [128, N] f32
    LTb = LT.astype(ml_dtypes.float8_e4m3)
    in_maps = []
    for d in range(NCORES):
        rows = slice(d * ROWS_PER_CORE, (d + 1) * ROWS_PER_CORE)
        qt_np = np.ascontiguousarray(
            (Q[rows] * SQRT_SCALE).T.astype(ml_dtypes.float8_e4m3))
        lw_np = np.empty((128, TILES_PER_CORE * W), dtype=ml_dtypes.float8_e4m3)
        zg_np = np.empty((128, TILES_PER_CORE * W), dtype=ml_dtypes.bfloat16)
        for r in range(TILES_PER_CORE):
            g = d * TILES_PER_CORE + r
            a = int(A[g])
            lw_np[:, r * W:(r + 1) * W] = LTb[:, a:a + W]
            eq = ads[a:a + W][None, :] == ads[g * 128:(g + 1) * 128][:, None]
            zg_np[:, r * W:(r + 1) * W] = np.where(eq, 0.0, GATE)
        in_maps.append({"qt": qt_np, "lt": LTb, "lw": lw_np, "zg": zg_np})
    return in_maps


def _make_corrections(ads, A, W):
    """Per-core [128, TILES] non-pair counts for the gate offset."""
    corr = []
    for d in range(NCORES):
        c = np.empty((128, TILES_PER_CORE), dtype=np.int64)
        for r in range(TILES_PER_CORE):
            g = d * TILES_PER_CORE + r
            a = int(A[g])
            eq = ads[a:a + W][None, :] == ads[g * 128:(g + 1) * 128][:, None]
            c[:, r] = W - eq.sum(axis=1)
        corr.append(c)
    return corr


def kernel(logits, labels, pad_mask, ad_idxs):
    logits_flat = np.ascontiguousarray(
        np.asarray(logits, dtype=np.float32).reshape(N, D))
    labels_flat = np.ascontiguousarray(
        np.asarray(labels, dtype=np.float32).reshape(N, D))
    valid = np.asarray(pad_mask).reshape(N) != 0
    ad = np.asarray(ad_idxs).reshape(N).astype(np.int64)

    if not valid.all():
        return _host_reference(logits_flat, labels_flat, valid, ad)

    prep = _prepare(logits_flat, labels_flat, ad)
    if prep is None:
        return _host_reference(logits_flat, labels_flat, valid, ad)
    order, ads, Q, L, p_total, W, A = prep

    nc = _get_program(W)
    in_maps = _make_in_maps(Q, L, ads, A, W)
    corr = _make_corrections(ads, A, W)

    from concourse import bass_utils
    res = bass_utils.run_bass_kernel_spmd(nc, in_maps, core_ids=list(range(NCORES)))
    s_scaled = 0.0
    for d, r in enumerate(res.results):
        bs = np.asarray(r["out"], dtype=np.float64)
        s_scaled += float(bs.sum()) - float(corr[d].sum()) * GATE
    loss = (C_BITS * p_total - s_scaled / 128.0) / N
    return np.float32(loss)
